# revision 1
# baseline (speedup 1.0000x reference)
"""Trainium2 Bass kernel for nn_EntityClassify (3-layer R-GCN over a
heterograph with node types a/b/d and 4 relations).

Strategy (8 NeuronCores, SPMD):
  - Dead-code pruning of the reference: the final output is only h3['d'],
    which transitively needs
        L0: h1_d = relu(mean_r0(feat_a) @ W0[0] + mean_r1(feat_b) @ W0[1] + b0)
        L1: h2_a = relu(mean_r2(h1_d) @ W1[2] + b1)
            h2_b = relu(mean_r3(h1_d) @ W1[3] + b1)
        L2: out  = mean_r0(h2_a) @ W2[0] + mean_r1(h2_b) @ W2[1] + b2
    (feat_d and all other relation weights are unused.)
  - Destination-node partitioning across the 8 cores; edges bucketed by
    (dst core, 128-row dst window) on the host, sorted by dst.
  - Per 128-edge chunk: one multi-row indirect DMA gathers the source rows
    into SBUF; a one-hot (dst one-hot scaled by 1/deg) mask is built with a
    single fused DVE op; TensorE matmul with the mask performs the
    segment-sum into PSUM (accumulated across a window's chunks).
  - mean-agg then transform (linear commutes with the segment mean).
  - L1 additionally fuses the layer-2 per-node transform (t = h2 @ W2[r])
    so only [N,16] tables (not [N,128]) are exchanged and gathered in L2.
  - AllGather collectives replicate h1_d and the t tables between layers.
"""

from contextlib import ExitStack

import numpy as np

P = 128
NCORES = 8

# Full-size problem config (hardcoded per the task contract).
CFG = dict(
    ND=50000, NA=100000, NB=100000,
    H=128, OUT=16, E=600000,
    DPC=6272,   # d-rows per core (49 windows of 128)
    APC=12544,  # a/b-rows per core (98 windows of 128)
)


def _ceil_div(a, b):
    return -(-a // b)


def _prep_relation(src, dst, rows_per_core, n_dst_real, n_cores=NCORES):
    """Bucket edges by (dst core, dst window), sort by dst, pad each window's
    edge list to a shared (max-over-cores) chunk count.

    Returns a schedule shared by all cores (K per window, chunk offsets) and
    per-core [128, T] arrays: src index (int32), dst-in-window (f32),
    1/deg (f32). Padding edges have w=0 so they contribute nothing.
    """
    src = np.asarray(src).astype(np.int64)
    dst = np.asarray(dst).astype(np.int64)
    deg = np.bincount(dst, minlength=n_dst_real)
    wnode = (1.0 / np.maximum(deg, 1.0)).astype(np.float32)

    order = np.argsort(dst, kind="stable")
    ssrc = src[order]
    sdst = dst[order]

    core = sdst // rows_per_core
    rem = sdst % rows_per_core
    win = rem // P
    n_win = rows_per_core // P

    cw = core * n_win + win
    counts = np.bincount(cw, minlength=n_cores * n_win)
    K = np.maximum(1, _ceil_div(counts.reshape(n_cores, n_win).max(axis=0), P))
    Koff = np.zeros(n_win + 1, np.int64)
    Koff[1:] = np.cumsum(K)
    T = int(Koff[-1])

    starts = np.zeros(n_cores * n_win + 1, np.int64)
    starts[1:] = np.cumsum(counts)
    rank = np.arange(len(sdst), dtype=np.int64) - starts[cw]
    pos = Koff[win] * P + rank  # position in the core's padded edge stream

    srcA = np.zeros((n_cores, T * P), np.int32)
    dstA = np.zeros((n_cores, T * P), np.float32)
    wA = np.zeros((n_cores, T * P), np.float32)
    srcA[core, pos] = ssrc.astype(np.int32)
    dstA[core, pos] = (rem % P).astype(np.float32)
    wA[core, pos] = wnode[sdst]

    def tp(a):
        # [T*P] stream -> [P, T]: column t is chunk t (one edge per partition)
        return np.ascontiguousarray(a.reshape(n_cores, T, P).transpose(0, 2, 1))

    return dict(
        K=[int(k) for k in K],
        Koff=[int(k) for k in Koff],
        T=T,
        src=tp(srcA),
        dst=tp(dstA),
        w=tp(wA),
    )


def preprocess(inputs, cfg=CFG):
    """Host-side: edge bucketing/sorting, basis->W einsum, constants."""
    inp = {k: np.asarray(v) for k, v in inputs.items()}
    H, OUT = cfg["H"], cfg["OUT"]

    R = {
        0: _prep_relation(inp["e0_src"], inp["e0_dst"], cfg["DPC"], cfg["ND"]),
        1: _prep_relation(inp["e1_src"], inp["e1_dst"], cfg["DPC"], cfg["ND"]),
        2: _prep_relation(inp["e2_src"], inp["e2_dst"], cfg["APC"], cfg["NA"]),
        3: _prep_relation(inp["e3_src"], inp["e3_dst"], cfg["APC"], cfg["NB"]),
    }

    W0 = np.einsum("rb,bio->rio", inp["coef0"], inp["basis0"]).astype(np.float32)
    W1 = np.einsum("rb,bio->rio", inp["coef1"], inp["basis1"]).astype(np.float32)
    W2 = np.einsum("rb,bio->rio", inp["coef2"], inp["basis2"]).astype(np.float32)

    common = {
        "w00": np.ascontiguousarray(W0[0]),
        "w01": np.ascontiguousarray(W0[1]),
        "w12": np.ascontiguousarray(W1[2]),
        "w13": np.ascontiguousarray(W1[3]),
        "w20": np.ascontiguousarray(W2[0]),
        "w21": np.ascontiguousarray(W2[1]),
        "bias0t": np.ascontiguousarray(
            np.broadcast_to(inp["bias0"].astype(np.float32), (P, H))
        ),
        "bias1c": np.ascontiguousarray(inp["bias1"].astype(np.float32)[:, None]),
        "bias2t": np.ascontiguousarray(
            np.broadcast_to(inp["bias2"].astype(np.float32), (P, OUT))
        ),
        "iota": np.ascontiguousarray(
            np.broadcast_to(np.arange(P, dtype=np.float32), (P, P))
        ),
        "feat_a": np.ascontiguousarray(inp["feat_a"].astype(np.float32)),
        "feat_b": np.ascontiguousarray(inp["feat_b"].astype(np.float32)),
    }

    in_maps = []
    for c in range(NCORES):
        m = dict(common)
        for r in range(4):
            m[f"r{r}_src"] = R[r]["src"][c]
            m[f"r{r}_dst"] = R[r]["dst"][c]
            m[f"r{r}_w"] = R[r]["w"][c]
        in_maps.append(m)

    sched = {r: dict(K=R[r]["K"], Koff=R[r]["Koff"], T=R[r]["T"]) for r in R}
    return sched, in_maps


def build_program(sched, cfg=CFG, phases=("L0", "AG1", "L1a", "AG2a", "L1b", "AG2b", "L2")):
    import concourse.bass as bass
    import concourse.mybir as mybir
    import concourse.tile as tile
    from concourse import bacc

    f32 = mybir.dt.float32
    i32 = mybir.dt.int32
    Alu = mybir.AluOpType
    Act = mybir.ActivationFunctionType

    H, OUT = cfg["H"], cfg["OUT"]
    n_win_d = cfg["DPC"] // P
    n_win_a = cfg["APC"] // P
    ND_PAD = NCORES * cfg["DPC"]
    NA_PAD = NCORES * cfg["APC"]
    RG = [list(range(NCORES))]

    nc = bacc.Bacc(
        "TRN2", target_bir_lowering=False, debug=False, num_devices=NCORES
    )

    feat_a = nc.dram_tensor("feat_a", [cfg["NA"], H], f32, kind="ExternalInput")
    feat_b = nc.dram_tensor("feat_b", [cfg["NB"], H], f32, kind="ExternalInput")
    meta_d = {}
    for r in range(4):
        T = sched[r]["T"]
        meta_d[r] = dict(
            src=nc.dram_tensor(f"r{r}_src", [P, T], i32, kind="ExternalInput"),
            dst=nc.dram_tensor(f"r{r}_dst", [P, T], f32, kind="ExternalInput"),
            w=nc.dram_tensor(f"r{r}_w", [P, T], f32, kind="ExternalInput"),
        )
    consts_spec = {
        "w00": [H, H], "w01": [H, H], "w12": [H, H], "w13": [H, H],
        "w20": [H, OUT], "w21": [H, OUT],
        "bias0t": [P, H], "bias1c": [P, 1], "bias2t": [P, OUT],
        "iota": [P, P],
    }
    const_d = {
        k: nc.dram_tensor(k, shape, f32, kind="ExternalInput")
        for k, shape in consts_spec.items()
    }
    out_d = nc.dram_tensor("out_d", [cfg["DPC"], OUT], f32, kind="ExternalOutput")

    h1_slice = nc.dram_tensor("h1_slice", [cfg["DPC"], H], f32)
    h1_full = nc.dram_tensor("h1_full", [ND_PAD, H], f32, addr_space="Shared")
    ta_slice = nc.dram_tensor("ta_slice", [cfg["APC"], OUT], f32)
    tb_slice = nc.dram_tensor("tb_slice", [cfg["APC"], OUT], f32)
    ta_full = nc.dram_tensor("ta_full", [NA_PAD, OUT], f32, addr_space="Shared")
    tb_full = nc.dram_tensor("tb_full", [NA_PAD, OUT], f32, addr_space="Shared")

    with tile.TileContext(nc) as tc, ExitStack() as ctx:
        sb = ctx.enter_context(tc.tile_pool(name="sb", bufs=1))
        ps = ctx.enter_context(tc.tile_pool(name="ps", bufs=1, space="PSUM"))

        # "touch" tiles: concentrate load-DMA waits onto single cheap DVE ops
        # so downstream DVE/PE instructions inherit the dependency via
        # same-engine program order (the ISA allows only ~2 sync waits on a
        # DVE instruction).
        touch_v = sb.tile([1, 1], f32, name="touch_v", tag="touch_v")
        touch_g = sb.tile([1, 1], f32, name="touch_g", tag="touch_g")

        def touch(t, engine="v"):
            eng = nc.vector if engine == "v" else nc.gpsimd
            dest = touch_v if engine == "v" else touch_g
            eng.tensor_copy(out=dest[:], in_=t[0:1, 0:1].bitcast(f32))

        # constants -> SBUF
        cs = {}
        for k, shape in consts_spec.items():
            t = sb.tile(shape, f32, name=f"c_{k}", tag=f"c_{k}")
            nc.sync.dma_start(out=t[:], in_=const_d[k][:, :])
            cs[k] = t

        # edge metadata -> SBUF (resident for the whole kernel)
        msb = {}
        for r in range(4):
            T = sched[r]["T"]
            e = {}
            for part, dt in (("src", i32), ("dst", f32), ("w", f32)):
                t = sb.tile([P, T], dt, name=f"m{r}_{part}", tag=f"m{r}_{part}")
                nc.sync.dma_start(out=t[:], in_=meta_d[r][part][:, :])
                e[part] = t
            msb[r] = e

        for t in cs.values():
            touch(t)
        for r in range(4):
            for part in ("dst", "w"):
                touch(msb[r][part])
            # src meta is consumed by the gather's descriptor generation on
            # gpsimd, so touch it there
            touch(msb[r]["src"], engine="g")

        def aggT_window(rel, w_i, table):
            """Segment-mean of gathered table rows for one 128-dst window.
            Returns SBUF tile aggT [H(in) x 128(dst)].

            NB: the HW indirect DGE consumes exactly one index per partition
            per instruction, so each 128-edge chunk is its own gather."""
            K = sched[rel]["K"][w_i]
            t0 = sched[rel]["Koff"][w_i]
            pA = ps.tile([P, P], f32, name="pA", tag="pA", bufs=3)
            for k in range(K):
                G = sb.tile([P, H], f32, name="G", tag="G", bufs=10)
                nc.gpsimd.indirect_dma_start(
                    out=G[:],
                    out_offset=None,
                    in_=table[:, :],
                    in_offset=bass.IndirectOffsetOnAxis(
                        ap=msb[rel]["src"][:, t0 + k:t0 + k + 1], axis=0
                    ),
                )
                mk = sb.tile([P, P], f32, name="mk", tag="mk", bufs=6)
                nc.vector.scalar_tensor_tensor(
                    out=mk[:],
                    in0=cs["iota"][:],
                    scalar=msb[rel]["dst"][:, t0 + k:t0 + k + 1],
                    in1=msb[rel]["w"][:, t0 + k:t0 + k + 1].to_broadcast([P, P]),
                    op0=Alu.is_equal,
                    op1=Alu.mult,
                )
                nc.tensor.matmul(
                    out=pA[:],
                    lhsT=G[:],
                    rhs=mk[:],
                    start=(k == 0),
                    stop=(k == K - 1),
                )
            a_sb = sb.tile([P, P], f32, name="aggT", tag="aggT", bufs=3)
            nc.vector.tensor_copy(out=a_sb[:], in_=pA[:])
            return a_sb

        # ---------------- Layer 0: h1_d ----------------
        with nc.named_scope("L0"):
            for w_i in range(n_win_d if "L0" in phases else 0):
                a0 = aggT_window(0, w_i, feat_a)
                a1 = aggT_window(1, w_i, feat_b)
                pB = ps.tile([P, H], f32, name="pB", tag="pB", bufs=2)
                nc.tensor.matmul(out=pB[:], lhsT=a0[:], rhs=cs["w00"][:],
                                 start=True, stop=False)
                nc.tensor.matmul(out=pB[:], lhsT=a1[:], rhs=cs["w01"][:],
                                 start=False, stop=True)
                tmp = sb.tile([P, H], f32, name="tmp", tag="tmp", bufs=3)
                nc.vector.tensor_tensor(out=tmp[:], in0=pB[:], in1=cs["bias0t"][:],
                                        op=Alu.add)
                h1sb = sb.tile([P, H], f32, name="h1sb", tag="h1sb", bufs=3)
                nc.vector.tensor_scalar_max(out=h1sb[:], in0=tmp[:], scalar1=0.0)
                nc.sync.dma_start(out=h1_slice[w_i * P:(w_i + 1) * P, :],
                                  in_=h1sb[:])

        with nc.named_scope("AG1"):
            if "AG1" in phases:
                nc.gpsimd.collective_compute(
                    "AllGather", mybir.AluOpType.bypass, replica_groups=RG,
                    ins=[h1_slice[:, :]], outs=[h1_full[:, :]],
                )

        # ---------------- Layer 1 (+ fused layer-2 transform) ----------------
        def l1_pass(rel, w1_t, w2_t, t_slice):
            for w_i in range(n_win_a):
                a_sb = aggT_window(rel, w_i, h1_full)
                pB = ps.tile([P, P], f32, name="pB2", tag="pB", bufs=2)
                # h2T [out x dst] so the bias lands on partitions
                nc.tensor.matmul(out=pB[:], lhsT=w1_t[:], rhs=a_sb[:],
                                 start=True, stop=True)
                h2T = sb.tile([P, P], f32, name="h2T", tag="h2T", bufs=3)
                nc.scalar.activation(out=h2T[:], in_=pB[:], func=Act.Relu,
                                     bias=cs["bias1c"][:], scale=1.0)
                pC = ps.tile([P, OUT], f32, name="pC", tag="pC", bufs=2)
                nc.tensor.matmul(out=pC[:], lhsT=h2T[:], rhs=w2_t[:],
                                 start=True, stop=True)
                tsb = sb.tile([P, OUT], f32, name="tsb", tag="tsb", bufs=3)
                nc.vector.tensor_copy(out=tsb[:], in_=pC[:])
                nc.sync.dma_start(out=t_slice[w_i * P:(w_i + 1) * P, :],
                                  in_=tsb[:])

        with nc.named_scope("L1a"):
            if "L1a" in phases:
                l1_pass(2, cs["w12"], cs["w20"], ta_slice)
        with nc.named_scope("AG2a"):
            if "AG2a" in phases:
                nc.gpsimd.collective_compute(
                    "AllGather", mybir.AluOpType.bypass, replica_groups=RG,
                    ins=[ta_slice[:, :]], outs=[ta_full[:, :]],
                )
        with nc.named_scope("L1b"):
            if "L1b" in phases:
                l1_pass(3, cs["w13"], cs["w21"], tb_slice)
        with nc.named_scope("AG2b"):
            if "AG2b" in phases:
                nc.gpsimd.collective_compute(
                    "AllGather", mybir.AluOpType.bypass, replica_groups=RG,
                    ins=[tb_slice[:, :]], outs=[tb_full[:, :]],
                )

        # ---------------- Layer 2: out_d ----------------
        with nc.named_scope("L2"):
            for w_i in range(n_win_d if "L2" in phases else 0):
                pC = ps.tile([P, OUT], f32, name="pC2", tag="pC", bufs=2)
                first = True
                for rel, tbl in ((0, ta_full), (1, tb_full)):
                    K = sched[rel]["K"][w_i]
                    t0 = sched[rel]["Koff"][w_i]
                    for k in range(K):
                        Gt = sb.tile([P, OUT], f32, name="Gt", tag="Gt", bufs=10)
                        nc.gpsimd.indirect_dma_start(
                            out=Gt[:],
                            out_offset=None,
                            in_=tbl[:, :],
                            in_offset=bass.IndirectOffsetOnAxis(
                                ap=msb[rel]["src"][:, t0 + k:t0 + k + 1], axis=0
                            ),
                        )
                        mk = sb.tile([P, P], f32, name="mk2", tag="mk", bufs=6)
                        nc.vector.scalar_tensor_tensor(
                            out=mk[:],
                            in0=cs["iota"][:],
                            scalar=msb[rel]["dst"][:, t0 + k:t0 + k + 1],
                            in1=msb[rel]["w"][:, t0 + k:t0 + k + 1].to_broadcast(
                                [P, P]
                            ),
                            op0=Alu.is_equal,
                            op1=Alu.mult,
                        )
                        last = (rel == 1 and k == K - 1)
                        nc.tensor.matmul(
                            out=pC[:],
                            lhsT=mk[:],
                            rhs=Gt[:],
                            start=first,
                            stop=last,
                        )
                        first = False
                osb = sb.tile([P, OUT], f32, name="osb", tag="osb", bufs=3)
                nc.vector.tensor_tensor(out=osb[:], in0=pC[:], in1=cs["bias2t"][:],
                                        op=Alu.add)
                nc.sync.dma_start(out=out_d[w_i * P:(w_i + 1) * P, :], in_=osb[:])

    return nc


LAST_RESULTS = None  # stashed BassKernelResults for test harnesses


def kernel(**inputs):
    global LAST_RESULTS
    from concourse.bass_utils import run_bass_kernel_spmd

    sched, in_maps = preprocess(inputs, CFG)
    nc = build_program(sched, CFG)
    nc.finalize()
    res = run_bass_kernel_spmd(nc, in_maps, list(range(NCORES)), trace=False)
    LAST_RESULTS = res
    out = np.concatenate([res.results[c]["out_d"] for c in range(NCORES)], axis=0)
    return np.ascontiguousarray(out[:CFG["ND"]].astype(np.float32))

